# revision 3
# baseline (speedup 1.0000x reference)
"""SLAYER SNN (fc -> psp -> spike, twice) Trainium2 Bass kernel.

Sharding: data-parallel over batch. 8 cores x 4 batches each; weights
replicated (host pre-transposed/packed). Input spikes are {0,1}, so fp8
staging is exact; W1 is scaled by 16 into the fp8-e4m3 sweet spot and
rescaled for free inside the qp activation.

v2: the whole post-L1 pipeline runs in fp8 with DoubleRow contraction
pairing (two 128-deep t-chunks per PE pass), roughly halving the psp /
refractory / psp-trick / W2 matmul column counts; the layer-2 spike
chain runs per-batch inline in [t, unit] layout (tiny 16-col ops that
hide under the next batch's L1 stream) instead of one serial tail, and
each batch's output rows DMA out as soon as they are ready.

Per-core pipeline (layer-1 runs in [t-on-partition] layout; layer 2 rides
on psp-matmul linearity -- psp(W2 s1) == W2 psp(s1)):
  z1T[t',o] : PE fp8 DoubleRow matmul -- input chunks stationary [k,2,t'],
              W1T moving [k,2,o]; 256-deep contraction per instruction
  z1Tq      : PSUM f32 -> fp8 SBUF cast (split ACT/DVE); exactness margin
              verified against the reference (|dp1| <= 0.5 < margin)
  p1T[t',o] : PE banded-Toeplitz matmul with the truncated SRM alpha
              kernel (fp8 taps), DoubleRow over t-chunk pairs
  s0T       : candidate spikes vs theta*16 straight out of PSUM (DVE)
  qpT       : (theta - p1T/16)  (ACT affine, folds the W1 x16 scale)
  wT[t',o]  : refractory response = K_ref-Toeplitz(s0T), fp8 DoubleRow
  s1T       : (wT >= qpT)  (DVE) -- vectorized refractory correction
  Ps1[o,t]  : psp(s1) via the transpose trick (stationary = s1T chunk
              pair, moving = K_psp row-block pair, DoubleRow)
  p2[16,t]  : PE fp8 DoubleRow matmul W2T-stationary (x8 scale) -- two
              256-deep passes over the hidden dim
  layer-2 spike chain per batch: [16,t]->[t,16] transposes, s0/qp on
              48-wide tiles, K_ref DoubleRow matmuls, s1, [t,16]->[16,t]
              transposes, f32 cast, early per-batch DMA out.

Orchestration for the ~8us DMA ramp + PE HAM clock-gate:
  - input DMA split fine-grained and need-ordered over three hw queues
  - batch 0's L1 runs ic-outer with two open PSUM accumulators at DMA
    pace, with dependency-free filler matmuls to un-throttle the HAM
  - post(b) interleaves into L1(b+1)'s matmul groups; the layer-2 chain
    of batch b is spread through batch b+1's post-chain slots so its
    cross-engine latency hides under PE streams
"""

import numpy as np
from contextlib import ExitStack

import concourse.bass as bass
import concourse.bacc as bacc
import concourse.tile as tile
import concourse.mybir as mybir
import concourse.bass_utils as bass_utils

F32 = mybir.dt.float32
BF16 = mybir.dt.bfloat16
FP8 = mybir.dt.float8e4
AF = mybir.ActivationFunctionType
OP = mybir.AluOpType
PM = mybir.MatmulPerfMode

B, NIN, NHID, NOUT, T = 32, 2312, 512, 10, 350
NCORES = 8
BL = B // NCORES            # 4 local batches per core
NIC2 = (NIN + 255) // 256   # 10 double-row contraction chunks
NIN_PAD = NIC2 * 256        # 2560
NOC = NHID // 128           # 4 hidden chunks
NTC = (T + 127) // 128      # 3 time chunks
T_PAD = NTC * 128           # 384
NO2 = 16                    # NOUT padded to 16 (fp8 DR step%16 rule)

THETA = 10.0
TS = 1.0
D_REF = float(np.exp(-TS / 1.0))          # refractory decay, tau_ref = 1
C_REF = float(-2.0 * THETA * np.e * TS / 1.0)
REF_TAPS = 30
W1SCALE = 16.0
W2SCALE = 8.0


def _srm_kernel():
    # mirrors reference._alpha_kernel truncation rule (tau=10, eps=0.01)
    ks = []
    for t in np.arange(0.0, T, TS):
        v = t / 10.0 * np.exp(1.0 - t / 10.0)
        if abs(v) < 0.01 and t > 10.0:
            break
        ks.append(v)
    return np.asarray(ks, dtype=np.float32)


def _toeplitz_mats():
    a = _srm_kernel()                       # 77 taps
    kp = np.zeros((T_PAD, T_PAD), np.float32)
    for j in range(len(a)):
        kp[np.arange(0, T - j), np.arange(j, T)] = a[j] * TS
    kr = np.zeros((T_PAD, T_PAD), np.float32)
    for j in range(1, REF_TAPS + 1):
        if j < T:
            kr[np.arange(0, T - j), np.arange(j, T)] = (
                C_REF * j * D_REF ** j)
    return kp, kr


def _kern(ctx, tc, xm, w1m, w2t, kp, kr, ident, out):
    nc = tc.nc
    singles = ctx.enter_context(tc.tile_pool(name="singles", bufs=1))
    xb_pool = ctx.enter_context(tc.tile_pool(name="xb", bufs=4))
    z1t_pool = ctx.enter_context(tc.tile_pool(name="z1t", bufs=2))
    qp_pool = ctx.enter_context(tc.tile_pool(name="qpp", bufs=4))
    s0_pool = ctx.enter_context(tc.tile_pool(name="s0p", bufs=2))
    s1t_pool = ctx.enter_context(tc.tile_pool(name="s1tp", bufs=2))
    ps1_pool = ctx.enter_context(tc.tile_pool(name="ps1", bufs=2))
    l2_pool = ctx.enter_context(tc.tile_pool(name="l2", bufs=2))
    z1psum = ctx.enter_context(tc.tile_pool(name="z1psum", bufs=3, space="PSUM"))
    pqsum = ctx.enter_context(tc.tile_pool(name="pqsum", bufs=3, space="PSUM"))
    pspsum = ctx.enter_context(tc.tile_pool(name="pspsum", bufs=2, space="PSUM"))

    # ---- constants + input DMA across the 3 hw queues (sync/SP,
    # scalar/Activation, gpsimd), fine-grained and need-ordered ----
    xb_b = [
        xb_pool.tile([128, NIC2, 2, T_PAD], FP8, name=f"xb{b}", tag="xb")
        for b in range(BL)]
    w1t_sb = singles.tile([128, NIC2, 2, NHID], FP8)
    kp_sb = singles.tile([128, NTC, T_PAD], FP8)
    kr_sb = singles.tile([128, NTC, T_PAD], FP8)
    w2t_sb = singles.tile([128, NOC, NO2], FP8)
    id_sb = singles.tile([128, 128], BF16)
    wu = singles.tile([128, NHID], BF16)
    nc.vector.memset(wu[:], 0.0)
    nc.vector.memset(kp_sb[:], 0.0)
    nc.vector.memset(kr_sb[:], 0.0)
    nc.sync.dma_start(w1t_sb[:, 0:2], w1m[:, 0:2])
    nc.sync.dma_start(xb_b[0][:, 2:4], xm[0, :, 2:4])
    nc.sync.dma_start(w1t_sb[:, 4:6], w1m[:, 4:6])
    nc.sync.dma_start(xb_b[0][:, 6:8], xm[0, :, 6:8])
    nc.sync.dma_start(xb_b[1][:, 0:5], xm[1, :, 0:5])
    nc.sync.dma_start(xb_b[2][:, 0:5], xm[2, :, 0:5])
    nc.scalar.dma_start(xb_b[0][:, 0:2], xm[0, :, 0:2])
    nc.scalar.dma_start(w1t_sb[:, 2:4], w1m[:, 2:4])
    nc.scalar.dma_start(xb_b[0][:, 4:6], xm[0, :, 4:6])
    nc.scalar.dma_start(w1t_sb[:, 6:8], w1m[:, 6:8])
    nc.scalar.dma_start(xb_b[1][:, 5:NIC2], xm[1, :, 5:NIC2])
    nc.scalar.dma_start(w2t_sb[:], w2t)
    nc.scalar.dma_start(id_sb[:], ident)
    nc.scalar.dma_start(xb_b[2][:, 5:NIC2], xm[2, :, 5:NIC2])
    nc.gpsimd.dma_start(xb_b[0][:, 8:NIC2], xm[0, :, 8:NIC2])
    nc.gpsimd.dma_start(w1t_sb[:, 8:NIC2], w1m[:, 8:NIC2])
    for tcn in range(NTC):
        hi = min(tcn * 128 + 208, T)
        nc.gpsimd.dma_start(kp_sb[:, tcn, tcn * 128 : hi],
                            kp[:, tcn, tcn * 128 : hi])
    for tcn in range(NTC):
        hi = min(tcn * 128 + 160, T)
        nc.gpsimd.dma_start(kr_sb[:, tcn, tcn * 128 : hi],
                            kr[:, tcn, tcn * 128 : hi])
    nc.gpsimd.dma_start(xb_b[3][:, 0:5], xm[3, :, 0:5])
    nc.gpsimd.dma_start(xb_b[3][:, 5:NIC2], xm[3, :, 5:NIC2])

    # ---- PE warm-up during the DMA lead-in (HAM un-throttle) ----
    for i in range(4):
        zw = z1psum.tile([128, NHID], F32, name=f"warm{i}", tag="zp")
        nc.tensor.matmul(zw[:, :], wu[:, 0:128], wu[:, :], start=True, stop=True)

    z1t_all = [None] * BL
    qp_all = [[None] * NTC for _ in range(BL)]
    s0_all = [None] * BL
    s1t_all = [None] * BL
    ps1_all = [None] * BL
    z2p_all = [None] * BL
    p2t_all = [None] * BL
    s02_all = [None] * BL
    qp2_all = [None] * BL
    s12_all = [None] * BL
    s2sb = singles.tile([128, T_PAD], F32)

    def _new_z1t(b):
        z1t_all[b] = z1t_pool.tile([128, NTC, NHID], FP8,
                                   name=f"z1t{b}", tag="z1t")

    def _cast_z1t(b, tpc, zp):
        if tpc == 1:
            nc.vector.tensor_copy(z1t_all[b][:, tpc, :], zp[:, :])
        else:
            nc.scalar.copy(z1t_all[b][:, tpc, :], zp[:, :])

    def l1_group(b, tpc):
        if tpc == 0:
            _new_z1t(b)
        zp = z1psum.tile([128, NHID], F32, name=f"zp{b}{tpc}", tag="zp")
        for ic in range(NIC2):
            nc.tensor.matmul(
                zp[:, :],
                xb_b[b][:, ic, :, tpc * 128 : (tpc + 1) * 128],
                w1t_sb[:, ic, :, :],
                start=(ic == 0), stop=(ic == NIC2 - 1),
                perf_mode=PM.DoubleRow)
        _cast_z1t(b, tpc, zp)

    def psp_tpc(b, tpc):
        if tpc == 0:
            s0_all[b] = s0_pool.tile([128, NTC, NHID], FP8,
                                     name=f"s0t{b}", tag="s0t")
        pp = pqsum.tile([128, NHID], F32, name=f"pp{b}{tpc}", tag="pq")
        if tpc == 0:
            nc.tensor.matmul(
                pp[:, :], kp_sb[:, 0, 0:128], z1t_all[b][:, 0, :],
                start=True, stop=True)
        else:
            nc.tensor.matmul(
                pp[:, :],
                kp_sb[:, tpc - 1 : tpc + 1, tpc * 128 : (tpc + 1) * 128],
                z1t_all[b][:, tpc - 1 : tpc + 1, :],
                start=True, stop=True, perf_mode=PM.DoubleRow)
        nc.vector.tensor_single_scalar(
            s0_all[b][:, tpc, :], pp[:, :], THETA * W1SCALE, OP.is_ge)
        qpt = qp_pool.tile([128, NHID], BF16, name=f"qpt{b}{tpc}", tag="qpt")
        nc.scalar.activation(qpt[:, :], pp[:, :], AF.Copy,
                             bias=THETA, scale=-1.0 / W1SCALE)
        qp_all[b][tpc] = qpt

    def ref_tpc(b, tpc):
        if tpc == 0:
            s1t_all[b] = s1t_pool.tile([128, NTC, NHID], FP8,
                                       name=f"s1t{b}", tag="s1t")
        wp = pqsum.tile([128, NHID], F32, name=f"wp{b}{tpc}", tag="pq")
        if tpc == 0:
            nc.tensor.matmul(
                wp[:, :], kr_sb[:, 0, 0:128], s0_all[b][:, 0, :],
                start=True, stop=True)
        else:
            nc.tensor.matmul(
                wp[:, :],
                kr_sb[:, tpc - 1 : tpc + 1, tpc * 128 : (tpc + 1) * 128],
                s0_all[b][:, tpc - 1 : tpc + 1, :],
                start=True, stop=True, perf_mode=PM.DoubleRow)
        nc.vector.tensor_tensor(
            s1t_all[b][:, tpc, :], wp[:, :], qp_all[b][tpc][:, :], OP.is_ge)

    def trick_l2(b):
        # Ps1[o, t] = psp(s1)[o, t]: stationary = s1T chunk pair, moving =
        # Kp row-block pair (DoubleRow, 256-deep t' contraction); chunk 2
        # rides a plain fp8 suffix pass. start=True clears the whole bank.
        ps1_all[b] = ps1_pool.tile([128, NOC, T_PAD], FP8,
                                   name=f"ps1{b}", tag="psb")
        for oc in range(NOC):
            pw = pspsum.tile([128, T_PAD], F32, name=f"pw{b}{oc}", tag="pw")
            nc.tensor.matmul(
                pw[:, :],
                s1t_all[b][:, 0:2, oc * 128 : (oc + 1) * 128],
                kp_sb[:, 0:2, :],
                start=True, stop=False, perf_mode=PM.DoubleRow)
            nc.tensor.matmul(
                pw[:, 256:T_PAD],
                s1t_all[b][:, 2, oc * 128 : (oc + 1) * 128],
                kp_sb[:, 2, 256:T_PAD],
                start=False, stop=True)
            if oc % 2 == 0:
                nc.scalar.copy(ps1_all[b][:, oc, :], pw[:, :])
            else:
                nc.vector.tensor_copy(ps1_all[b][:, oc, :], pw[:, :])

    def z2_mm(b):
        # p2*W2SCALE = W2s @ psp(s1): two 256-deep fp8 DR passes
        z2p = pspsum.tile([NO2, T_PAD], F32, name=f"z2p{b}", tag="pw")
        for h in range(2):
            nc.tensor.matmul(
                z2p[:, :],
                w2t_sb[:, 2 * h : 2 * h + 2, :],
                ps1_all[b][:, 2 * h : 2 * h + 2, :],
                start=(h == 0), stop=(h == 1), perf_mode=PM.DoubleRow)
        z2p_all[b] = z2p

    # ---- layer-2 spike chain, split into schedulable pieces ----
    def chain_a(b):
        # p2 [16,t] -> SBUF -> [t,16] transposes -> SBUF
        p2sb = l2_pool.tile([NO2, T_PAD], BF16, name=f"p2sb{b}", tag="p2sb")
        nc.scalar.copy(p2sb[:, :], z2p_all[b][:, :])
        p2ps = pqsum.tile([128, NTC, NO2], BF16, name=f"p2ps{b}", tag="pq")
        for tc_ in range(NTC):
            nc.tensor.transpose(
                p2ps[:, tc_, :],
                p2sb[:, tc_ * 128 : (tc_ + 1) * 128],
                id_sb[0:NO2, 0:NO2])
        p2t = l2_pool.tile([128, NTC, NO2], BF16, name=f"p2t{b}", tag="p2t")
        nc.vector.tensor_copy(p2t[:, :, :], p2ps[:, :, :])
        p2t_all[b] = p2t

    def chain_b(b):
        # candidate spikes + qp on the [t, unit] tiles, then K_ref matmuls
        s02 = l2_pool.tile([128, NTC, NO2], FP8, name=f"s02{b}", tag="s02")
        nc.vector.tensor_single_scalar(
            s02[:, :, :], p2t_all[b][:, :, :], THETA * W2SCALE, OP.is_ge)
        qp2 = l2_pool.tile([128, NTC, NO2], BF16, name=f"qp2{b}", tag="qp2")
        nc.scalar.activation(qp2[:, :, :], p2t_all[b][:, :, :], AF.Copy,
                             bias=THETA, scale=-1.0 / W2SCALE)
        s02_all[b] = s02
        qp2_all[b] = qp2

    def chain_c(b):
        s12 = l2_pool.tile([128, NTC, NO2], BF16, name=f"s12{b}", tag="s12")
        for tc_ in range(NTC):
            w2p = pqsum.tile([128, NO2], F32, name=f"w2p{b}{tc_}", tag="pq")
            if tc_ == 0:
                nc.tensor.matmul(
                    w2p[:, :], kr_sb[:, 0, 0:128], s02_all[b][:, 0, :],
                    start=True, stop=True)
            else:
                nc.tensor.matmul(
                    w2p[:, :],
                    kr_sb[:, tc_ - 1 : tc_ + 1, tc_ * 128 : (tc_ + 1) * 128],
                    s02_all[b][:, tc_ - 1 : tc_ + 1, :],
                    start=True, stop=True, perf_mode=PM.DoubleRow)
            nc.vector.tensor_tensor(
                s12[:, tc_, :], w2p[:, :], qp2_all[b][:, tc_, :], OP.is_ge)
        s12_all[b] = s12

    def chain_d(b, q):
        # [t,16] -> [16,t] transposes, f32 cast into the packed output
        # rows, then this batch's 10 output rows DMA out immediately
        for tc_ in range(NTC):
            s2f = pspsum.tile([NO2, 128], BF16, name=f"s2f{b}{tc_}", tag="pw")
            nc.tensor.transpose(s2f[:, :], s12_all[b][:, tc_, :], id_sb[:])
            if tc_ % 2 == 0:
                nc.scalar.copy(
                    s2sb[b * 32 : b * 32 + NO2,
                         tc_ * 128 : (tc_ + 1) * 128], s2f[:, :])
            else:
                nc.vector.tensor_copy(
                    s2sb[b * 32 : b * 32 + NO2,
                         tc_ * 128 : (tc_ + 1) * 128], s2f[:, :])
        q.dma_start(out[b * NOUT : (b + 1) * NOUT, :],
                    s2sb[b * 32 : b * 32 + NOUT, :T])

    # ---- software pipeline ----
    # batch 0 streams: consume each (x, w1) chunk pair in BOTH open
    # t-chunk accumulators the moment it lands (PE at DMA pace, HAM warm)
    _new_z1t(0)
    zp00 = z1psum.tile([128, NHID], F32, name="zp00", tag="zp")
    zp01 = z1psum.tile([128, NHID], F32, name="zp01", tag="zp")
    for ic in range(NIC2):
        for tpc, zp in ((0, zp00), (1, zp01)):
            nc.tensor.matmul(
                zp[:, :],
                xb_b[0][:, ic, :, tpc * 128 : (tpc + 1) * 128],
                w1t_sb[:, ic, :, :],
                start=(ic == 0), stop=(ic == NIC2 - 1),
                perf_mode=PM.DoubleRow)
        if 1 <= ic <= NIC2 - 2:
            for j in range(3):
                fz = pqsum.tile([128, NHID], F32, name=f"fil{ic}{j}", tag="pq")
                nc.tensor.matmul(fz[:, 0:256], wu[:, 0:128], wu[:, 0:256],
                                 start=True, stop=True)
    _cast_z1t(0, 0, zp00)
    _cast_z1t(0, 1, zp01)

    l1_group(0, 2)
    # batch 0's post runs BEFORE L1(1): post-chain PE bubbles hide under
    # the still-streaming batch 1-2 DMA
    for t in range(NTC):
        psp_tpc(0, t)
    l1_group(1, 0)
    for t in range(NTC):
        ref_tpc(0, t)
    l1_group(1, 1)
    trick_l2(0)
    l1_group(1, 2)
    z2_mm(0)
    # batch b's layer-2 chain pieces are spread through batch b+1's
    # post-chain so every cross-engine hop hides under a PE stream
    for b in (1, 2):
        nb = b + 1
        l1_group(nb, 0)
        chain_a(b - 1)
        for t in range(NTC):
            psp_tpc(b, t)
        chain_b(b - 1)
        l1_group(nb, 1)
        chain_c(b - 1)
        for t in range(NTC):
            ref_tpc(b, t)
        l1_group(nb, 2)
        chain_d(b - 1, nc.sync if b == 1 else nc.scalar)
        if nb == BL - 1:
            psp_tpc(BL - 1, 0)
            psp_tpc(BL - 1, 1)
        trick_l2(b)
        z2_mm(b)
    psp_tpc(BL - 1, 2)
    chain_a(2)
    for t in range(NTC):
        ref_tpc(BL - 1, t)
    chain_b(2)
    trick_l2(BL - 1)
    chain_c(2)
    z2_mm(BL - 1)
    chain_d(2, nc.gpsimd)
    chain_a(3)
    chain_b(3)
    chain_c(3)
    chain_d(3, nc.sync)


def build():
    nc = bacc.Bacc("TRN2", target_bir_lowering=False, debug=False,
                   enable_asserts=False, num_devices=NCORES)
    xm = nc.dram_tensor("x_in", [BL, 128, NIC2, 2, T_PAD], FP8,
                        kind="ExternalInput").ap()
    w1m = nc.dram_tensor("w1t", [128, NIC2, 2, NHID], FP8,
                         kind="ExternalInput").ap()
    w2t = nc.dram_tensor("w2t", [128, NOC, NO2], FP8, kind="ExternalInput").ap()
    kp = nc.dram_tensor("kp", [128, NTC, T_PAD], FP8, kind="ExternalInput").ap()
    kr = nc.dram_tensor("kr", [128, NTC, T_PAD], FP8, kind="ExternalInput").ap()
    ident = nc.dram_tensor("ident", [128, 128], BF16, kind="ExternalInput").ap()
    out = nc.dram_tensor("s2_out", [BL * NOUT, T], F32, kind="ExternalOutput").ap()
    with tile.TileContext(nc) as tc:
        with ExitStack() as ctx:
            _kern(ctx, tc, xm, w1m, w2t, kp, kr, ident, out)
    nc.compile()
    return nc


_CACHE = {}


def _get_nc():
    if "nc" not in _CACHE:
        _CACHE["nc"] = build()
    return _CACHE["nc"]


def _pack_kc(a, nchunk):
    # [(nchunk*128), X] -> [128, nchunk, X]  (partition-contiguous staging)
    return np.ascontiguousarray(
        a.reshape(nchunk, 128, a.shape[-1]).transpose(1, 0, 2))


def _make_in_maps(spikeInput, W1, W2):
    import ml_dtypes
    f8 = ml_dtypes.float8_e4m3
    bf = ml_dtypes.bfloat16
    xs = np.zeros((B, NIN_PAD, T_PAD), dtype=f8)
    xs[:, :NIN, :T] = spikeInput.astype(f8)
    # [B, (c k two), t] -> [B, k, c, two, t]
    xs = np.ascontiguousarray(
        xs.reshape(B, NIC2, 128, 2, T_PAD).transpose(0, 2, 1, 3, 4))
    w1t = np.zeros((NIN_PAD, NHID), dtype=f8)
    w1t[:NIN, :] = (W1.T * W1SCALE).astype(f8)
    w1t = np.ascontiguousarray(
        w1t.reshape(NIC2, 128, 2, NHID).transpose(1, 0, 2, 3))
    w2t = np.zeros((NHID, NO2), np.float32)
    w2t[:, :NOUT] = W2.T * W2SCALE
    w2t = _pack_kc(w2t.astype(f8), NOC)
    kpf, krf = _toeplitz_mats()
    kpb = _pack_kc(kpf.astype(f8), NTC)
    krb = _pack_kc(krf.astype(f8), NTC)
    ident = np.eye(128, dtype=bf)
    return [
        {"x_in": xs[c * BL : (c + 1) * BL], "w1t": w1t, "w2t": w2t,
         "kp": kpb, "kr": krb, "ident": ident}
        for c in range(NCORES)
    ]


def run(spikeInput, W1, W2, trace=False):
    nc = _get_nc()
    res = bass_utils.run_bass_kernel_spmd(
        nc, _make_in_maps(spikeInput, W1, W2),
        core_ids=list(range(NCORES)), trace=trace)
    out = np.empty((B, NOUT, T), np.float32)
    for c in range(NCORES):
        out[c * BL : (c + 1) * BL] = res.results[c]["s2_out"].reshape(BL, NOUT, T)
    return out, res


def kernel(spikeInput, W1, W2):
    out, _ = run(np.asarray(spikeInput), np.asarray(W1), np.asarray(W2))
    return out
